# revision 20
# baseline (speedup 1.0000x reference)
"""Trainium2 Bass kernel for nn_Attention_7945689497706.

v5 structure:
- K=64 row-tiled sim matmul pairs (both heads of a pair concurrently on
  PE row groups 0:64 / 64:128 — no zero padding).
- Attention as one flat 68-round stream per batch: round r issues the
  sim pair of (sweep r//8, jc r%8), the av matmuls of round r-2, and the
  sweep epilogues lagged so no PE instruction heads the queue waiting.
- Weights kept in f32r, DMA'd straight into SBUF: gamma folds into xn
  (scalar_tensor_tensor), the q-scale dh^-0.5 folds into exp's scale.
- Each batch self-fills its q/k/v projections and the other batch's
  work into its own exp-bound attention bubbles as 2-MM units.
"""

import numpy as np

import concourse.bass as bass
import concourse.mybir as mybir
import concourse.tile as tile
from concourse import bacc
from concourse.bass_utils import run_bass_kernel_spmd

F32 = mybir.dt.float32
F32R = mybir.dt.float32r
BF16 = mybir.dt.bfloat16
AF = mybir.ActivationFunctionType

NCORES = 8
B = 16
C = 512
N = 1024          # pixels = 32*32
HEADS = 8
DH = 64
NMEM = 4
PB = B // NCORES  # batch elements per core
CT = C // 128     # channel partition-tiles
NPAIR = HEADS // 2
VW = HEADS * (DH + 1)  # vext width: per head [v | ones] = 65
QSC = DH ** -0.5


def _build():
    nc = bacc.Bacc()
    x_ext = nc.declare_dram_parameter("x", [PB, C, N], F32, isOutput=False)
    wqkvt_ext = nc.declare_dram_parameter("wqkvt", [C, 3 * C], F32, isOutput=False)
    wot_ext = nc.declare_dram_parameter("wot", [C, C], F32, isOutput=False)
    gammat_ext = nc.declare_dram_parameter("gammat", [128, CT], F32, isOutput=False)
    memk_ext = nc.declare_dram_parameter("memk", [128, NPAIR, NMEM], F32, isOutput=False)
    memv_ext = nc.declare_dram_parameter("memv", [128, 2, VW], F32, isOutput=False)
    out_ext = nc.declare_dram_parameter("out", [PB, C, N], F32, isOutput=True)

    with tile.TileContext(nc) as tc:
        with (
            tc.tile_pool(name="const", bufs=1) as const,
            tc.tile_pool(name="wstage", bufs=1) as wstage,
            tc.tile_pool(name="wqs", bufs=2) as wqs,
            tc.tile_pool(name="xp", bufs=2) as xp,
            tc.tile_pool(name="data", bufs=1) as data,
            tc.tile_pool(name="atp", bufs=2) as atp,
            tc.tile_pool(name="qp", bufs=2) as qp,
            tc.tile_pool(name="pp", bufs=4) as pp,
            tc.tile_pool(name="pm", bufs=2) as pm,
            tc.tile_pool(name="avs", bufs=2) as avsp,
            tc.tile_pool(name="rp", bufs=2) as rp,
            tc.tile_pool(name="ob", bufs=3) as obp,
            tc.tile_pool(name="qkv_ps", bufs=2, space="PSUM") as qkv_ps,
            tc.tile_pool(name="sim_ps", bufs=2, space="PSUM") as sim_ps,
            tc.tile_pool(name="av_ps", bufs=2, space="PSUM") as av_ps,
        ):
            # ---------------- DMA staging ----------------
            xraws = []
            for bb in range(PB):
                xr = xp.tile([128, CT, N], F32, tag="xraw")
                xraws.append(xr)

            wqkv = const.tile([128, CT, 3 * C], BF16, tag="wqkv")
            wo = const.tile([128, CT, C], BF16, tag="wo")
            g1 = const.tile([128, CT], F32, tag="g1")
            ones1 = const.tile([128, 64], F32R, tag="ones1")
            ones128 = const.tile([128, 128], BF16, tag="ones128")
            # kT packed per head-pair: rows 0:64 = even head (d), 64:128 = odd
            kTp = const.tile([128, NPAIR, 1028], BF16, tag="kTp")
            vextA = const.tile([128, 8, VW], BF16, tag="vextA")
            vextB = const.tile([128, 8, VW], BF16, tag="vextB")
            vmem = const.tile([128, 2, VW], BF16, tag="vmem")
            gsb = const.tile([128, CT], F32, tag="gsb")
            vexts = [vextA, vextB]

            # sync queue: x0 then the qkv weights (needed first)
            for t in range(2):
                nc.sync.dma_start(out=xraws[0][:, t, :], in_=x_ext[0, t * 128:(t + 1) * 128, :])
            for t in range(CT):
                wq = wqs.tile([128, 3 * C], F32, tag="wq")
                nc.sync.dma_start(out=wq, in_=wqkvt_ext[t * 128:(t + 1) * 128, :])
                nc.gpsimd.tensor_copy(out=wqkv[:, t, :], in_=wq)
            # scalar queue: rest of x0, gamma, mem consts, then x1
            for t in range(2, CT):
                nc.scalar.dma_start(out=xraws[0][:, t, :], in_=x_ext[0, t * 128:(t + 1) * 128, :])
            nc.scalar.dma_start(out=gsb, in_=gammat_ext[:, :])
            ws = wstage.tile([128, NPAIR * NMEM + 2 * VW], F32, tag="ws")
            nc.scalar.dma_start(out=ws[:, 0:NPAIR * NMEM],
                                in_=memk_ext[:, :, :].rearrange("p g c -> p (g c)"))
            nc.scalar.dma_start(out=ws[:, NPAIR * NMEM:NPAIR * NMEM + 2 * VW],
                                in_=memv_ext[:, :, :].rearrange("p g c -> p (g c)"))
            for t in range(CT):
                nc.scalar.dma_start(out=xraws[1][:, t, :], in_=x_ext[1, t * 128:(t + 1) * 128, :])
            # out-proj weights are only needed late; keep them last on sync
            wostages = []
            for t in range(CT):
                wst = wstage.tile([128, C], F32, tag="wst" + str(t))
                nc.sync.dma_start(out=wst, in_=wot_ext[t * 128:(t + 1) * 128, :])
                wostages.append(wst)

            def wo_cast_unit(t):
                return lambda: nc.gpsimd.tensor_copy(out=wo[:, t, :], in_=wostages[t])

            nc.scalar.activation(out=g1, in_=gsb, func=AF.Copy, bias=1.0)
            nc.vector.memset(ones128, 1.0)
            nc.vector.memset(ones1.bitcast(F32), 1.0)
            nc.vector.tensor_copy(
                out=kTp[:, :, 1024:1028],
                in_=ws[:, 0:NPAIR * NMEM].rearrange("p (g c) -> p g c", c=NMEM))
            nc.vector.tensor_copy(
                out=vmem,
                in_=ws[:, NPAIR * NMEM:NPAIR * NMEM + 2 * VW].rearrange("p (g c) -> p g c", c=VW))
            for v in vexts:
                oc = v[:, :, :].rearrange("p j (h c) -> p j h c", c=DH + 1)[:, :, :, DH:DH + 1]
                nc.gpsimd.memset(oc, 1.0)

            # ---------------- pipeline stages ----------------
            def norm(bb):
                """x -> xn = x * (gamma+1) / rms(x)  (bf16)."""
                xraw = xraws[bb]
                xsq = data.tile([128, CT, N], BF16, tag="xsq")
                for t in range(CT):
                    nc.vector.tensor_mul(out=xsq[:, t, :], in0=xraw[:, t, :], in1=xraw[:, t, :])
                ss = sim_ps.tile([128, N], F32, tag="sim")
                for h2 in range(2):
                    for t in range(CT):
                        nc.tensor.matmul(ss[:, h2 * 512:(h2 + 1) * 512], ones128,
                                         xsq[:, t, h2 * 512:(h2 + 1) * 512],
                                         start=(t == 0), stop=(t == CT - 1))
                sroot = data.tile([128, N], F32, tag="sroot")
                nc.scalar.activation(out=sroot, in_=ss, func=AF.Sqrt, scale=1.0 / C)
                snorm = data.tile([128, N], F32, tag="snorm")
                nc.vector.reciprocal_approx_fast(out=snorm, in_=sroot)
                xn = data.tile([128, CT, N], BF16, tag="xn" + str(bb))
                for t in range(CT):
                    nc.vector.scalar_tensor_tensor(
                        out=xn[:, t, :], in0=xraw[:, t, :], scalar=g1[:, t:t + 1],
                        in1=snorm, op0=mybir.AluOpType.mult, op1=mybir.AluOpType.mult)
                return xn

            def qkproj(xn, qT, mcs):
                for mc in mcs:
                    for h2 in range(2):
                        ps = qkv_ps.tile([128, 512], F32, tag="q")
                        for t in range(CT):
                            nc.tensor.matmul(ps, wqkv[:, t, mc * 128:(mc + 1) * 128],
                                             xn[:, t, h2 * 512:(h2 + 1) * 512],
                                             start=(t == 0), stop=(t == CT - 1))
                        if mc < 4:
                            nc.vector.tensor_copy(out=qT[:, mc, h2 * 512:(h2 + 1) * 512], in_=ps)
                        else:
                            nc.vector.tensor_copy(
                                out=kTp[:, mc - 4, h2 * 512:(h2 + 1) * 512], in_=ps)

            def qkproj_units(xn, qT, mc):
                state = {}
                units = []
                for h2 in range(2):
                    def u1(h2=h2):
                        ps = qkv_ps.tile([128, 512], F32, tag="q")
                        state[h2] = ps
                        for t in (0, 1):
                            nc.tensor.matmul(ps, wqkv[:, t, mc * 128:(mc + 1) * 128],
                                             xn[:, t, h2 * 512:(h2 + 1) * 512],
                                             start=(t == 0), stop=False)

                    def u2(h2=h2):
                        ps = state[h2]
                        for t in (2, 3):
                            nc.tensor.matmul(ps, wqkv[:, t, mc * 128:(mc + 1) * 128],
                                             xn[:, t, h2 * 512:(h2 + 1) * 512],
                                             start=False, stop=(t == CT - 1))
                        if mc < 4:
                            nc.vector.tensor_copy(out=qT[:, mc, h2 * 512:(h2 + 1) * 512], in_=ps)
                        else:
                            nc.vector.tensor_copy(
                                out=kTp[:, mc - 4, h2 * 512:(h2 + 1) * 512], in_=ps)
                    units.append(u1)
                    units.append(u2)
                return units

            def vproj(xn, vext, ics):
                for ic in ics:
                    ps = qkv_ps.tile([128, 512], F32, tag="q")
                    for t in range(CT):
                        nc.tensor.matmul(ps, xn[:, t, ic * 128:(ic + 1) * 128],
                                         wqkv[:, t, 2 * C:3 * C],
                                         start=(t == 0), stop=(t == CT - 1))
                    ps_h = ps[:, :].rearrange("p (h c) -> p h c", c=DH)
                    vdst = vext[:, ic, :].rearrange("p (h c) -> p h c", c=DH + 1)[:, :, 0:DH]
                    nc.vector.tensor_copy(out=vdst, in_=ps_h)

            def vproj_units(xn, vext, ic):
                state = {}

                def u1():
                    ps = qkv_ps.tile([128, 512], F32, tag="q")
                    state[0] = ps
                    for t in (0, 1):
                        nc.tensor.matmul(ps, xn[:, t, ic * 128:(ic + 1) * 128],
                                         wqkv[:, t, 2 * C:3 * C],
                                         start=(t == 0), stop=False)

                def u2():
                    ps = state[0]
                    for t in (2, 3):
                        nc.tensor.matmul(ps, xn[:, t, ic * 128:(ic + 1) * 128],
                                         wqkv[:, t, 2 * C:3 * C],
                                         start=False, stop=(t == CT - 1))
                    ps_h = ps[:, :].rearrange("p (h c) -> p h c", c=DH)
                    vdst = vext[:, ic, :].rearrange("p (h c) -> p h c", c=DH + 1)[:, :, 0:DH]
                    nc.vector.tensor_copy(out=vdst, in_=ps_h)
                return [u1, u2]

            def proj(attn, bb, mcs=None, h2s=(0, 1)):
                for mc in (range(CT) if mcs is None else mcs):
                    for h2 in h2s:
                        ps = qkv_ps.tile([128, 512], F32, tag="q")
                        for t in range(CT):
                            nc.tensor.matmul(ps, wo[:, t, mc * 128:(mc + 1) * 128],
                                             attn[:, t, h2 * 512:(h2 + 1) * 512],
                                             start=(t == 0), stop=(t == CT - 1))
                        ob = obp.tile([128, 512], F32, tag="ob")
                        nc.vector.tensor_copy(out=ob, in_=ps)
                        nc.sync.dma_start(
                            out=out_ext[bb, mc * 128:(mc + 1) * 128, h2 * 512:(h2 + 1) * 512],
                            in_=ob)

            def proj_units(attn, bb, mc, h2):
                state = {}

                def u1():
                    ps = qkv_ps.tile([128, 512], F32, tag="q")
                    state[0] = ps
                    for t in (0, 1):
                        nc.tensor.matmul(ps, wo[:, t, mc * 128:(mc + 1) * 128],
                                         attn[:, t, h2 * 512:(h2 + 1) * 512],
                                         start=(t == 0), stop=False)

                def u2():
                    ps = state[0]
                    for t in (2, 3):
                        nc.tensor.matmul(ps, wo[:, t, mc * 128:(mc + 1) * 128],
                                         attn[:, t, h2 * 512:(h2 + 1) * 512],
                                         start=False, stop=(t == CT - 1))
                    ob = obp.tile([128, 512], F32, tag="ob")
                    nc.vector.tensor_copy(out=ob, in_=ps)
                    nc.sync.dma_start(
                        out=out_ext[bb, mc * 128:(mc + 1) * 128, h2 * 512:(h2 + 1) * 512],
                        in_=ob)
                return [u1, u2]

            def memsim(qT, pmems, g):
                st = sim_ps.tile([128, N], F32, tag="sim")
                for h4 in range(4):
                    h = 4 * g + h4
                    p, hh = h // 2, h % 2
                    for h2 in range(2):
                        nc.tensor.matmul(
                            st[32 * h4:32 * h4 + NMEM, h2 * 512:(h2 + 1) * 512],
                            kTp[64 * hh:64 * hh + 64, p, 1024:1028],
                            qT[64 * hh:64 * hh + 64, p, h2 * 512:(h2 + 1) * 512],
                            start=True, stop=True, tile_position=(64 * hh, 32 * h4))
                pmt = pm.tile([128, N], BF16, tag="pm")
                nc.scalar.activation(out=pmt, in_=st, func=AF.Exp, scale=QSC)
                pmems[g] = pmt

            def memsim_unit(qT, pmems, g):
                return lambda: memsim(qT, pmems, g)

            def attention(qT, vext, attn, pmems, fill):
                """All 8 heads as one flat stream of 68 rounds."""
                state = {}
                avbs = {}
                pts = [None] * 64

                def sim_round(r):
                    s, jc = r // 8, r % 8
                    p, h2 = s // 2, s % 2
                    st = sim_ps.tile([128, N], F32, tag="sim")
                    for hh in range(2):
                        nc.tensor.matmul(
                            st[:, hh * 512:(hh + 1) * 512],
                            kTp[64 * hh:64 * hh + 64, p, jc * 128:(jc + 1) * 128],
                            qT[64 * hh:64 * hh + 64, p, h2 * 512:(h2 + 1) * 512],
                            start=True, stop=True)
                    pt = pp.tile([128, N], BF16, tag="p")
                    nc.scalar.activation(out=pt, in_=st, func=AF.Exp, scale=QSC)
                    pts[r] = pt

                def av_round(r):
                    s, jc = r // 8, r % 8
                    p, h2 = s // 2, s % 2
                    if jc == 0:
                        avA = av_ps.tile([65, 512], F32, tag="av")
                        avB = av_ps.tile([65, 512], F32, tag="av")
                        state[s] = (avA, avB)
                    avt = state[s]
                    for hh in range(2):
                        h = 2 * p + hh
                        nc.tensor.matmul(
                            avt[hh], vext[:, jc, h * (DH + 1):(h + 1) * (DH + 1)],
                            pts[r][:, hh * 512:(hh + 1) * 512],
                            start=(jc == 0), stop=False)

                def epilogue_a(s):
                    """mem-kv contribution + PSUM evacuation (frees av tiles)."""
                    p, h2 = s // 2, s % 2
                    avt = state.pop(s)
                    for hh in range(2):
                        h = 2 * p + hh
                        g, r0 = h // 4, 32 * (h % 4)
                        nc.tensor.matmul(
                            avt[hh],
                            vmem[r0:r0 + NMEM, g, (h % 4) * (DH + 1):(h % 4 + 1) * (DH + 1)],
                            pmems[g][r0:r0 + NMEM, h2 * 512:(h2 + 1) * 512],
                            start=False, stop=True, tile_position=(r0, 0))
                    pair = []
                    for hh in range(2):
                        avb = avsp.tile([65, 512], F32R, tag="avs")
                        with tc.high_priority(offset=64):
                            nc.vector.tensor_copy(out=avb, in_=avt[hh])
                        pair.append(avb)
                    avbs[s] = pair

                def epilogue_b(s):
                    """normalize: attn = av[0:64] / av[64], two rounds later
                    so bc never heads the PE queue waiting for the copy."""
                    p, h2 = s // 2, s % 2
                    for hh in range(2):
                        avb = avbs[s][hh]
                        bc = qkv_ps.tile([64, 512], F32, tag="q")
                        nc.tensor.matmul(bc, ones1[64:65, :], avb[64:65, :], start=True, stop=True)
                        rcp = rp.tile([64, 512], F32, tag="rcp")
                        nc.vector.reciprocal_approx_fast(out=rcp, in_=bc)
                        nc.vector.tensor_mul(
                            out=attn[64 * hh:64 * hh + 64, p, h2 * 512:(h2 + 1) * 512],
                            in0=avb[0:64, :].bitcast(F32), in1=rcp)

                for r in range(68):
                    if r < 64:
                        sim_round(r)
                    if 2 <= r:
                        ar = r - 2
                        if ar < 64:
                            av_round(ar)
                        if ar % 8 == 7 and ar // 8 < 8:
                            epilogue_a(ar // 8)
                    if 4 <= r:
                        br = r - 4
                        if br % 8 == 7 and br // 8 < 8:
                            epilogue_b(br // 8)
                    for u in (fill[r] if r < len(fill) else []):
                        u()

            # ---------------- schedule ----------------
            # Prologue: only what batch-0's first sweeps need.
            xn0 = norm(0)
            qT0 = qp.tile([128, CT, N], BF16, tag="qT")
            qT1 = qp.tile([128, CT, N], BF16, tag="qT")
            qkproj(xn0, qT0, [0, 1, 4])      # q pairs 0,1 + k pair 0
            vproj(xn0, vexts[0], [0, 1, 2, 3])
            pmem0 = [None, None]
            memsim(qT0, pmem0, 0)            # heads 0-3 (needs q0, q1)
            xn1 = norm(1)

            attn0 = atp.tile([128, CT, N], BF16, tag="attn")
            attn1 = atp.tile([128, CT, N], BF16, tag="attn")
            pmem1 = [None, None]

            def place(fill, r0, units, per_round=1):
                r, i = r0, 0
                while i < len(units):
                    for _ in range(per_round):
                        if i < len(units):
                            fill[r].append(units[i])
                            i += 1
                    r += 1

            # batch-0 attention fill: own v4-v7, q2-q3, mem group 1,
            # k1-k3, then batch-1's q/k0/v and its mem sims.
            # kTp pair i is last read by batch-0 at round 8*(2i+1)+7.
            f0 = [[] for _ in range(68)]
            place(f0, 0, vproj_units(xn0, vexts[0], 4)
                  + vproj_units(xn0, vexts[0], 5)
                  + vproj_units(xn0, vexts[0], 6)
                  + vproj_units(xn0, vexts[0], 7), per_round=2)
            place(f0, 4, qkproj_units(xn0, qT0, 2))    # q pair 2
            place(f0, 8, qkproj_units(xn0, qT0, 3))    # q pair 3
            place(f0, 12, [memsim_unit(qT0, pmem0, 1)])  # heads 4-7, by rnd 41
            place(f0, 13, qkproj_units(xn0, qT0, 5))   # k1, read from rnd 16
            place(f0, 17, qkproj_units(xn0, qT0, 6))   # k2, read from rnd 32
            place(f0, 21, qkproj_units(xn0, qT0, 7))   # k3, read from rnd 48
            place(f0, 25, qkproj_units(xn1, qT1, 0))
            place(f0, 29, qkproj_units(xn1, qT1, 1))
            place(f0, 33, qkproj_units(xn1, qT1, 4))   # batch-1 k0 (safe: >15)
            place(f0, 37, qkproj_units(xn1, qT1, 2))
            place(f0, 41, qkproj_units(xn1, qT1, 3))
            place(f0, 45, [u for ic in range(8) for u in vproj_units(xn1, vexts[1], ic)])
            place(f0, 57, [wo_cast_unit(t) for t in range(CT)])
            attention(qT0, vexts[0], attn0, pmem0, f0)

            # batch-1 attention fill: own k1-k3, then batch-0 out-proj.
            f1 = [[] for _ in range(68)]
            place(f1, 0, [memsim_unit(qT1, pmem1, 0), memsim_unit(qT1, pmem1, 1)])
            place(f1, 2, qkproj_units(xn1, qT1, 5))
            place(f1, 6, qkproj_units(xn1, qT1, 6))
            place(f1, 10, qkproj_units(xn1, qT1, 7))
            place(f1, 14, [u for mc in range(CT) for h2 in range(2)
                           for u in proj_units(attn0, 0, mc, h2)])
            attention(qT1, vexts[1], attn1, pmem1, f1)
            proj(attn1, 1)
    nc.compile()
    return nc


_NC_CACHE = []


def kernel(x, gamma, mem_kv, w_qkv, w_out, _trace=False):
    x = np.asarray(x, dtype=np.float32)
    gamma = np.asarray(gamma, dtype=np.float32)
    mem_kv = np.asarray(mem_kv, dtype=np.float32)
    w_qkv = np.asarray(w_qkv, dtype=np.float32)
    w_out = np.asarray(w_out, dtype=np.float32)

    b, c, hh, ww = x.shape
    n = hh * ww
    xs = x.reshape(b, c, n)

    wqkvt = np.ascontiguousarray(w_qkv.T)          # [c, 3c]
    wot = np.ascontiguousarray(w_out.T)            # [c, c]
    gammat = np.ascontiguousarray(gamma.reshape(CT, 128).T)  # [128, CT]

    memk = np.zeros((128, NPAIR, NMEM), np.float32)
    memv = np.zeros((128, 2, VW), np.float32)
    for h in range(HEADS):
        p, hh_ = h // 2, h % 2
        memk[64 * hh_:64 * hh_ + DH, p, 0:NMEM] = mem_kv[0, h].T  # [dh, nmem]
        g, r1, c0 = h // 4, 32 * (h % 4), (h % 4) * (DH + 1)
        memv[r1:r1 + NMEM, g, c0:c0 + DH] = mem_kv[1, h]
        memv[r1:r1 + NMEM, g, c0 + DH] = 1.0

    if not _NC_CACHE:
        _NC_CACHE.append(_build())
    nc = _NC_CACHE[0]

    in_maps = []
    for core in range(NCORES):
        in_maps.append({
            "x": np.ascontiguousarray(xs[core * PB:(core + 1) * PB]),
            "wqkvt": wqkvt,
            "wot": wot,
            "gammat": gammat,
            "memk": memk,
            "memv": memv,
        })
    res = run_bass_kernel_spmd(nc, in_maps, core_ids=list(range(NCORES)), trace=_trace)
    out = np.concatenate([res.results[core]["out"] for core in range(NCORES)], axis=0)
    kernel.last_result = res
    return out.reshape(b, c, hh, ww)


# revision 21
# speedup vs baseline: 1.0775x; 1.0775x over previous
"""Trainium2 Bass kernel for nn_Attention_7945689497706.

v5 structure:
- K=64 row-tiled sim matmul pairs (both heads of a pair concurrently on
  PE row groups 0:64 / 64:128 — no zero padding).
- Attention as one flat 68-round stream per batch: round r issues the
  sim pair of (sweep r//8, jc r%8), the av matmuls of round r-2, and the
  sweep epilogues lagged so no PE instruction heads the queue waiting.
- Weights kept in f32r, DMA'd straight into SBUF: gamma folds into xn
  (scalar_tensor_tensor), the q-scale dh^-0.5 folds into exp's scale.
- Each batch self-fills its q/k/v projections and the other batch's
  work into its own exp-bound attention bubbles as 2-MM units.
"""

import numpy as np

import concourse.bass as bass
import concourse.mybir as mybir
import concourse.tile as tile
from concourse import bacc
from concourse.bass_utils import run_bass_kernel_spmd

F32 = mybir.dt.float32
F32R = mybir.dt.float32r
BF16 = mybir.dt.bfloat16
AF = mybir.ActivationFunctionType

NCORES = 8
B = 16
C = 512
N = 1024          # pixels = 32*32
HEADS = 8
DH = 64
NMEM = 4
PB = B // NCORES  # batch elements per core
CT = C // 128     # channel partition-tiles
NPAIR = HEADS // 2
VW = HEADS * (DH + 1)  # vext width: per head [v | ones] = 65
QSC = DH ** -0.5


def _build():
    nc = bacc.Bacc()
    x_ext = nc.declare_dram_parameter("x", [PB, C, N], F32, isOutput=False)
    wqkvt_ext = nc.declare_dram_parameter("wqkvt", [C, 3 * C], F32, isOutput=False)
    wot_ext = nc.declare_dram_parameter("wot", [C, C], F32, isOutput=False)
    gammat_ext = nc.declare_dram_parameter("gammat", [128, CT], F32, isOutput=False)
    memk_ext = nc.declare_dram_parameter("memk", [128, NPAIR, NMEM], F32, isOutput=False)
    memv_ext = nc.declare_dram_parameter("memv", [128, 2, VW], F32, isOutput=False)
    out_ext = nc.declare_dram_parameter("out", [PB, C, N], F32, isOutput=True)

    with tile.TileContext(nc) as tc:
        with (
            tc.tile_pool(name="const", bufs=1) as const,
            tc.tile_pool(name="wstage", bufs=1) as wstage,
            tc.tile_pool(name="wqs", bufs=2) as wqs,
            tc.tile_pool(name="xp", bufs=2) as xp,
            tc.tile_pool(name="data", bufs=1) as data,
            tc.tile_pool(name="atp", bufs=2) as atp,
            tc.tile_pool(name="qp", bufs=2) as qp,
            tc.tile_pool(name="pp", bufs=4) as pp,
            tc.tile_pool(name="pm", bufs=2) as pm,
            tc.tile_pool(name="avs", bufs=2) as avsp,
            tc.tile_pool(name="rp", bufs=2) as rp,
            tc.tile_pool(name="ob", bufs=3) as obp,
            tc.tile_pool(name="qkv_ps", bufs=2, space="PSUM") as qkv_ps,
            tc.tile_pool(name="sim_ps", bufs=2, space="PSUM") as sim_ps,
            tc.tile_pool(name="av_ps", bufs=2, space="PSUM") as av_ps,
        ):
            # ---------------- DMA staging ----------------
            xraws = []
            for bb in range(PB):
                xr = xp.tile([128, CT, N], F32, tag="xraw")
                xraws.append(xr)

            wqkv = const.tile([128, CT, 3 * C], BF16, tag="wqkv")
            wo = const.tile([128, CT, C], BF16, tag="wo")
            g1 = const.tile([128, CT], F32, tag="g1")
            ones1 = const.tile([128, 64], F32R, tag="ones1")
            ones128 = const.tile([128, 128], BF16, tag="ones128")
            # kT packed per head-pair: rows 0:64 = even head (d), 64:128 = odd
            kTp = const.tile([128, NPAIR, 1028], BF16, tag="kTp")
            vextA = const.tile([128, 8, VW], BF16, tag="vextA")
            vextB = const.tile([128, 8, VW], BF16, tag="vextB")
            vmem = const.tile([128, 2, VW], BF16, tag="vmem")
            gsb = const.tile([128, CT], F32, tag="gsb")
            vexts = [vextA, vextB]

            # sync queue: x0 then the qkv weights (needed first)
            for t in range(2):
                nc.sync.dma_start(out=xraws[0][:, t, :], in_=x_ext[0, t * 128:(t + 1) * 128, :])
            for t in range(CT):
                wq = wqs.tile([128, 3 * C], F32, tag="wq")
                nc.sync.dma_start(out=wq, in_=wqkvt_ext[t * 128:(t + 1) * 128, :])
                nc.vector.tensor_copy(out=wqkv[:, t, :], in_=wq)
            # scalar queue: rest of x0, gamma, mem consts, then x1
            for t in range(2, CT):
                nc.scalar.dma_start(out=xraws[0][:, t, :], in_=x_ext[0, t * 128:(t + 1) * 128, :])
            nc.scalar.dma_start(out=gsb, in_=gammat_ext[:, :])
            ws = wstage.tile([128, NPAIR * NMEM + 2 * VW], F32, tag="ws")
            nc.scalar.dma_start(out=ws[:, 0:NPAIR * NMEM],
                                in_=memk_ext[:, :, :].rearrange("p g c -> p (g c)"))
            nc.scalar.dma_start(out=ws[:, NPAIR * NMEM:NPAIR * NMEM + 2 * VW],
                                in_=memv_ext[:, :, :].rearrange("p g c -> p (g c)"))
            for t in range(CT):
                nc.scalar.dma_start(out=xraws[1][:, t, :], in_=x_ext[1, t * 128:(t + 1) * 128, :])
            # out-proj weights are only needed late; keep them last on sync
            wostages = []
            for t in range(CT):
                wst = wstage.tile([128, C], F32, tag="wst" + str(t))
                nc.sync.dma_start(out=wst, in_=wot_ext[t * 128:(t + 1) * 128, :])
                wostages.append(wst)

            def wo_cast_unit(t):
                return lambda: nc.vector.tensor_copy(out=wo[:, t, :], in_=wostages[t])

            nc.scalar.activation(out=g1, in_=gsb, func=AF.Copy, bias=1.0)
            nc.vector.memset(ones128, 1.0)
            nc.vector.memset(ones1.bitcast(F32), 1.0)
            nc.vector.tensor_copy(
                out=kTp[:, :, 1024:1028],
                in_=ws[:, 0:NPAIR * NMEM].rearrange("p (g c) -> p g c", c=NMEM))
            nc.vector.tensor_copy(
                out=vmem,
                in_=ws[:, NPAIR * NMEM:NPAIR * NMEM + 2 * VW].rearrange("p (g c) -> p g c", c=VW))
            for v in vexts:
                oc = v[:, :, :].rearrange("p j (h c) -> p j h c", c=DH + 1)[:, :, :, DH:DH + 1]
                nc.gpsimd.memset(oc, 1.0)

            # ---------------- pipeline stages ----------------
            def norm(bb):
                """x -> xn = x * (gamma+1) / rms(x)  (bf16)."""
                xraw = xraws[bb]
                xsq = data.tile([128, CT, N], BF16, tag="xsq")
                for t in range(CT):
                    nc.vector.tensor_mul(out=xsq[:, t, :], in0=xraw[:, t, :], in1=xraw[:, t, :])
                ss = sim_ps.tile([128, N], F32, tag="sim")
                for h2 in range(2):
                    for t in range(CT):
                        nc.tensor.matmul(ss[:, h2 * 512:(h2 + 1) * 512], ones128,
                                         xsq[:, t, h2 * 512:(h2 + 1) * 512],
                                         start=(t == 0), stop=(t == CT - 1))
                sroot = data.tile([128, N], F32, tag="sroot")
                nc.scalar.activation(out=sroot, in_=ss, func=AF.Sqrt, scale=1.0 / C)
                snorm = data.tile([128, N], F32, tag="snorm")
                nc.vector.reciprocal_approx_fast(out=snorm, in_=sroot)
                xn = data.tile([128, CT, N], BF16, tag="xn" + str(bb))
                for t in range(CT):
                    nc.vector.scalar_tensor_tensor(
                        out=xn[:, t, :], in0=xraw[:, t, :], scalar=g1[:, t:t + 1],
                        in1=snorm, op0=mybir.AluOpType.mult, op1=mybir.AluOpType.mult)
                return xn

            def qkproj(xn, qT, mcs):
                for mc in mcs:
                    for h2 in range(2):
                        ps = qkv_ps.tile([128, 512], F32, tag="q")
                        for t in range(CT):
                            nc.tensor.matmul(ps, wqkv[:, t, mc * 128:(mc + 1) * 128],
                                             xn[:, t, h2 * 512:(h2 + 1) * 512],
                                             start=(t == 0), stop=(t == CT - 1))
                        if mc < 4:
                            nc.vector.tensor_copy(out=qT[:, mc, h2 * 512:(h2 + 1) * 512], in_=ps)
                        else:
                            nc.vector.tensor_copy(
                                out=kTp[:, mc - 4, h2 * 512:(h2 + 1) * 512], in_=ps)

            def qkproj_units(xn, qT, mc):
                state = {}
                units = []
                for h2 in range(2):
                    def u1(h2=h2):
                        ps = qkv_ps.tile([128, 512], F32, tag="q")
                        state[h2] = ps
                        for t in (0, 1):
                            nc.tensor.matmul(ps, wqkv[:, t, mc * 128:(mc + 1) * 128],
                                             xn[:, t, h2 * 512:(h2 + 1) * 512],
                                             start=(t == 0), stop=False)

                    def u2(h2=h2):
                        ps = state[h2]
                        for t in (2, 3):
                            nc.tensor.matmul(ps, wqkv[:, t, mc * 128:(mc + 1) * 128],
                                             xn[:, t, h2 * 512:(h2 + 1) * 512],
                                             start=False, stop=(t == CT - 1))
                        if mc < 4:
                            nc.vector.tensor_copy(out=qT[:, mc, h2 * 512:(h2 + 1) * 512], in_=ps)
                        else:
                            nc.vector.tensor_copy(
                                out=kTp[:, mc - 4, h2 * 512:(h2 + 1) * 512], in_=ps)
                    units.append(u1)
                    units.append(u2)
                return units

            def vproj(xn, vext, ics):
                for ic in ics:
                    ps = qkv_ps.tile([128, 512], F32, tag="q")
                    for t in range(CT):
                        nc.tensor.matmul(ps, xn[:, t, ic * 128:(ic + 1) * 128],
                                         wqkv[:, t, 2 * C:3 * C],
                                         start=(t == 0), stop=(t == CT - 1))
                    ps_h = ps[:, :].rearrange("p (h c) -> p h c", c=DH)
                    vdst = vext[:, ic, :].rearrange("p (h c) -> p h c", c=DH + 1)[:, :, 0:DH]
                    nc.vector.tensor_copy(out=vdst, in_=ps_h)

            def vproj_units(xn, vext, ic):
                state = {}

                def u1():
                    ps = qkv_ps.tile([128, 512], F32, tag="q")
                    state[0] = ps
                    for t in (0, 1):
                        nc.tensor.matmul(ps, xn[:, t, ic * 128:(ic + 1) * 128],
                                         wqkv[:, t, 2 * C:3 * C],
                                         start=(t == 0), stop=False)

                def u2():
                    ps = state[0]
                    for t in (2, 3):
                        nc.tensor.matmul(ps, xn[:, t, ic * 128:(ic + 1) * 128],
                                         wqkv[:, t, 2 * C:3 * C],
                                         start=False, stop=(t == CT - 1))
                    ps_h = ps[:, :].rearrange("p (h c) -> p h c", c=DH)
                    vdst = vext[:, ic, :].rearrange("p (h c) -> p h c", c=DH + 1)[:, :, 0:DH]
                    nc.vector.tensor_copy(out=vdst, in_=ps_h)
                return [u1, u2]

            def proj(attn, bb, mcs=None, h2s=(0, 1)):
                for mc in (range(CT) if mcs is None else mcs):
                    for h2 in h2s:
                        ps = qkv_ps.tile([128, 512], F32, tag="q")
                        for t in range(CT):
                            nc.tensor.matmul(ps, wo[:, t, mc * 128:(mc + 1) * 128],
                                             attn[:, t, h2 * 512:(h2 + 1) * 512],
                                             start=(t == 0), stop=(t == CT - 1))
                        ob = obp.tile([128, 512], F32, tag="ob")
                        nc.vector.tensor_copy(out=ob, in_=ps)
                        nc.sync.dma_start(
                            out=out_ext[bb, mc * 128:(mc + 1) * 128, h2 * 512:(h2 + 1) * 512],
                            in_=ob)

            def proj_units(attn, bb, mc, h2):
                state = {}

                def u1():
                    ps = qkv_ps.tile([128, 512], F32, tag="q")
                    state[0] = ps
                    for t in (0, 1):
                        nc.tensor.matmul(ps, wo[:, t, mc * 128:(mc + 1) * 128],
                                         attn[:, t, h2 * 512:(h2 + 1) * 512],
                                         start=(t == 0), stop=False)

                def u2():
                    ps = state[0]
                    for t in (2, 3):
                        nc.tensor.matmul(ps, wo[:, t, mc * 128:(mc + 1) * 128],
                                         attn[:, t, h2 * 512:(h2 + 1) * 512],
                                         start=False, stop=(t == CT - 1))
                    ob = obp.tile([128, 512], F32, tag="ob")
                    nc.vector.tensor_copy(out=ob, in_=ps)
                    nc.sync.dma_start(
                        out=out_ext[bb, mc * 128:(mc + 1) * 128, h2 * 512:(h2 + 1) * 512],
                        in_=ob)
                return [u1, u2]

            def memsim(qT, pmems, g):
                st = sim_ps.tile([128, N], F32, tag="sim")
                for h4 in range(4):
                    h = 4 * g + h4
                    p, hh = h // 2, h % 2
                    for h2 in range(2):
                        nc.tensor.matmul(
                            st[32 * h4:32 * h4 + NMEM, h2 * 512:(h2 + 1) * 512],
                            kTp[64 * hh:64 * hh + 64, p, 1024:1028],
                            qT[64 * hh:64 * hh + 64, p, h2 * 512:(h2 + 1) * 512],
                            start=True, stop=True, tile_position=(64 * hh, 32 * h4))
                pmt = pm.tile([128, N], BF16, tag="pm")
                nc.scalar.activation(out=pmt, in_=st, func=AF.Exp, scale=QSC)
                pmems[g] = pmt

            def memsim_unit(qT, pmems, g):
                return lambda: memsim(qT, pmems, g)

            def attention(qT, vext, attn, pmems, fill):
                """All 8 heads as one flat stream of 68 rounds."""
                state = {}
                avbs = {}
                pts = [None] * 64

                def sim_round(r):
                    s, jc = r // 8, r % 8
                    p, h2 = s // 2, s % 2
                    st = sim_ps.tile([128, N], F32, tag="sim")
                    for hh in range(2):
                        nc.tensor.matmul(
                            st[:, hh * 512:(hh + 1) * 512],
                            kTp[64 * hh:64 * hh + 64, p, jc * 128:(jc + 1) * 128],
                            qT[64 * hh:64 * hh + 64, p, h2 * 512:(h2 + 1) * 512],
                            start=True, stop=True)
                    pt = pp.tile([128, N], BF16, tag="p")
                    nc.scalar.activation(out=pt, in_=st, func=AF.Exp, scale=QSC)
                    pts[r] = pt

                def av_round(r):
                    s, jc = r // 8, r % 8
                    p, h2 = s // 2, s % 2
                    if jc == 0:
                        avA = av_ps.tile([65, 512], F32, tag="av")
                        avB = av_ps.tile([65, 512], F32, tag="av")
                        state[s] = (avA, avB)
                    avt = state[s]
                    for hh in range(2):
                        h = 2 * p + hh
                        nc.tensor.matmul(
                            avt[hh], vext[:, jc, h * (DH + 1):(h + 1) * (DH + 1)],
                            pts[r][:, hh * 512:(hh + 1) * 512],
                            start=(jc == 0), stop=False)

                def epilogue_a(s):
                    """mem-kv contribution + PSUM evacuation (frees av tiles)."""
                    p, h2 = s // 2, s % 2
                    avt = state.pop(s)
                    for hh in range(2):
                        h = 2 * p + hh
                        g, r0 = h // 4, 32 * (h % 4)
                        nc.tensor.matmul(
                            avt[hh],
                            vmem[r0:r0 + NMEM, g, (h % 4) * (DH + 1):(h % 4 + 1) * (DH + 1)],
                            pmems[g][r0:r0 + NMEM, h2 * 512:(h2 + 1) * 512],
                            start=False, stop=True, tile_position=(r0, 0))
                    pair = []
                    for hh in range(2):
                        avb = avsp.tile([65, 512], F32R, tag="avs")
                        with tc.high_priority(offset=64):
                            nc.vector.tensor_copy(out=avb, in_=avt[hh])
                        pair.append(avb)
                    avbs[s] = pair

                def epilogue_b(s):
                    """normalize: attn = av[0:64] / av[64], two rounds later
                    so bc never heads the PE queue waiting for the copy."""
                    p, h2 = s // 2, s % 2
                    for hh in range(2):
                        avb = avbs[s][hh]
                        bc = qkv_ps.tile([64, 512], F32, tag="q")
                        nc.tensor.matmul(bc, ones1[64:65, :], avb[64:65, :], start=True, stop=True)
                        rcp = rp.tile([64, 512], F32, tag="rcp")
                        nc.vector.reciprocal_approx_fast(out=rcp, in_=bc)
                        nc.vector.tensor_mul(
                            out=attn[64 * hh:64 * hh + 64, p, h2 * 512:(h2 + 1) * 512],
                            in0=avb[0:64, :].bitcast(F32), in1=rcp)

                for r in range(68):
                    if r < 64:
                        sim_round(r)
                    if 2 <= r:
                        ar = r - 2
                        if ar < 64:
                            av_round(ar)
                        if ar % 8 == 7 and ar // 8 < 8:
                            epilogue_a(ar // 8)
                    if 4 <= r:
                        br = r - 4
                        if br % 8 == 7 and br // 8 < 8:
                            epilogue_b(br // 8)
                    for u in (fill[r] if r < len(fill) else []):
                        u()

            # ---------------- schedule ----------------
            # Prologue: only what batch-0's first sweeps need.
            xn0 = norm(0)
            qT0 = qp.tile([128, CT, N], BF16, tag="qT")
            qT1 = qp.tile([128, CT, N], BF16, tag="qT")
            qkproj(xn0, qT0, [0, 1, 4])      # q pairs 0,1 + k pair 0
            vproj(xn0, vexts[0], [0, 1, 2, 3])
            pmem0 = [None, None]
            memsim(qT0, pmem0, 0)            # heads 0-3 (needs q0, q1)
            xn1 = norm(1)

            attn0 = atp.tile([128, CT, N], BF16, tag="attn")
            attn1 = atp.tile([128, CT, N], BF16, tag="attn")
            pmem1 = [None, None]

            def place(fill, r0, units, per_round=1):
                r, i = r0, 0
                while i < len(units):
                    for _ in range(per_round):
                        if i < len(units):
                            fill[r].append(units[i])
                            i += 1
                    r += 1

            # batch-0 attention fill: own v4-v7, q2-q3, mem group 1,
            # k1-k3, then batch-1's q/k0/v and its mem sims.
            # kTp pair i is last read by batch-0 at round 8*(2i+1)+7.
            f0 = [[] for _ in range(68)]
            place(f0, 0, vproj_units(xn0, vexts[0], 4)
                  + vproj_units(xn0, vexts[0], 5)
                  + vproj_units(xn0, vexts[0], 6)
                  + vproj_units(xn0, vexts[0], 7), per_round=2)
            place(f0, 4, qkproj_units(xn0, qT0, 2))    # q pair 2
            place(f0, 8, qkproj_units(xn0, qT0, 3))    # q pair 3
            place(f0, 12, [memsim_unit(qT0, pmem0, 1)])  # heads 4-7, by rnd 41
            place(f0, 13, qkproj_units(xn0, qT0, 5))   # k1, read from rnd 16
            place(f0, 17, qkproj_units(xn0, qT0, 6))   # k2, read from rnd 32
            place(f0, 21, qkproj_units(xn0, qT0, 7))   # k3, read from rnd 48
            place(f0, 25, qkproj_units(xn1, qT1, 0))
            place(f0, 29, qkproj_units(xn1, qT1, 1))
            place(f0, 33, qkproj_units(xn1, qT1, 4))   # batch-1 k0 (safe: >15)
            place(f0, 37, qkproj_units(xn1, qT1, 2))
            place(f0, 41, qkproj_units(xn1, qT1, 3))
            place(f0, 45, [u for ic in range(8) for u in vproj_units(xn1, vexts[1], ic)])
            place(f0, 57, [wo_cast_unit(t) for t in range(CT)])
            attention(qT0, vexts[0], attn0, pmem0, f0)

            # batch-1 attention fill: own k1-k3, then batch-0 out-proj.
            f1 = [[] for _ in range(68)]
            place(f1, 0, [memsim_unit(qT1, pmem1, 0), memsim_unit(qT1, pmem1, 1)])
            place(f1, 2, qkproj_units(xn1, qT1, 5))
            place(f1, 6, qkproj_units(xn1, qT1, 6))
            place(f1, 10, qkproj_units(xn1, qT1, 7))
            place(f1, 14, [u for mc in range(CT) for h2 in range(2)
                           for u in proj_units(attn0, 0, mc, h2)])
            attention(qT1, vexts[1], attn1, pmem1, f1)
            proj(attn1, 1)
    nc.compile()
    return nc


_NC_CACHE = []


def kernel(x, gamma, mem_kv, w_qkv, w_out, _trace=False):
    x = np.asarray(x, dtype=np.float32)
    gamma = np.asarray(gamma, dtype=np.float32)
    mem_kv = np.asarray(mem_kv, dtype=np.float32)
    w_qkv = np.asarray(w_qkv, dtype=np.float32)
    w_out = np.asarray(w_out, dtype=np.float32)

    b, c, hh, ww = x.shape
    n = hh * ww
    xs = x.reshape(b, c, n)

    wqkvt = np.ascontiguousarray(w_qkv.T)          # [c, 3c]
    wot = np.ascontiguousarray(w_out.T)            # [c, c]
    gammat = np.ascontiguousarray(gamma.reshape(CT, 128).T)  # [128, CT]

    memk = np.zeros((128, NPAIR, NMEM), np.float32)
    memv = np.zeros((128, 2, VW), np.float32)
    for h in range(HEADS):
        p, hh_ = h // 2, h % 2
        memk[64 * hh_:64 * hh_ + DH, p, 0:NMEM] = mem_kv[0, h].T  # [dh, nmem]
        g, r1, c0 = h // 4, 32 * (h % 4), (h % 4) * (DH + 1)
        memv[r1:r1 + NMEM, g, c0:c0 + DH] = mem_kv[1, h]
        memv[r1:r1 + NMEM, g, c0 + DH] = 1.0

    if not _NC_CACHE:
        _NC_CACHE.append(_build())
    nc = _NC_CACHE[0]

    in_maps = []
    for core in range(NCORES):
        in_maps.append({
            "x": np.ascontiguousarray(xs[core * PB:(core + 1) * PB]),
            "wqkvt": wqkvt,
            "wot": wot,
            "gammat": gammat,
            "memk": memk,
            "memv": memv,
        })
    res = run_bass_kernel_spmd(nc, in_maps, core_ids=list(range(NCORES)), trace=_trace)
    out = np.concatenate([res.results[core]["out"] for core in range(NCORES)], axis=0)
    kernel.last_result = res
    return out.reshape(b, c, hh, ww)


# revision 23
# speedup vs baseline: 1.2379x; 1.1489x over previous
"""Trainium2 Bass kernel for nn_Attention_7945689497706.

v5 structure:
- K=64 row-tiled sim matmul pairs (both heads of a pair concurrently on
  PE row groups 0:64 / 64:128 — no zero padding).
- Attention as one flat 68-round stream per batch: round r issues the
  sim pair of (sweep r//8, jc r%8), the av matmuls of round r-2, and the
  sweep epilogues lagged so no PE instruction heads the queue waiting.
- Weights kept in f32r, DMA'd straight into SBUF: gamma folds into xn
  (scalar_tensor_tensor), the q-scale dh^-0.5 folds into exp's scale.
- Each batch self-fills its q/k/v projections and the other batch's
  work into its own exp-bound attention bubbles as 2-MM units.
"""

import numpy as np

import concourse.bass as bass
import concourse.mybir as mybir
import concourse.tile as tile
from concourse import bacc
from concourse.bass_utils import run_bass_kernel_spmd

F32 = mybir.dt.float32
F32R = mybir.dt.float32r
BF16 = mybir.dt.bfloat16
AF = mybir.ActivationFunctionType

NCORES = 8
B = 16
C = 512
N = 1024          # pixels = 32*32
HEADS = 8
DH = 64
NMEM = 4
PB = B // NCORES  # batch elements per core
CT = C // 128     # channel partition-tiles
NPAIR = HEADS // 2
VW = HEADS * (DH + 1)  # vext width: per head [v | ones] = 65
QSC = DH ** -0.5


def _build():
    nc = bacc.Bacc()
    x_ext = nc.declare_dram_parameter("x", [PB, C, N], F32, isOutput=False)
    wqkvt_ext = nc.declare_dram_parameter("wqkvt", [C, 3 * C], F32, isOutput=False)
    wot_ext = nc.declare_dram_parameter("wot", [C, C], F32, isOutput=False)
    gammat_ext = nc.declare_dram_parameter("gammat", [128, CT], F32, isOutput=False)
    memk_ext = nc.declare_dram_parameter("memk", [128, NPAIR, NMEM], F32, isOutput=False)
    memv_ext = nc.declare_dram_parameter("memv", [128, 2, VW], F32, isOutput=False)
    out_ext = nc.declare_dram_parameter("out", [PB, C, N], F32, isOutput=True)

    with tile.TileContext(nc) as tc:
        with (
            tc.tile_pool(name="const", bufs=1) as const,
            tc.tile_pool(name="wstage", bufs=1) as wstage,
            tc.tile_pool(name="wqs", bufs=2) as wqs,
            tc.tile_pool(name="xp", bufs=2) as xp,
            tc.tile_pool(name="data", bufs=1) as data,
            tc.tile_pool(name="atp", bufs=2) as atp,
            tc.tile_pool(name="qp", bufs=2) as qp,
            tc.tile_pool(name="pp", bufs=4) as pp,
            tc.tile_pool(name="pm", bufs=2) as pm,
            tc.tile_pool(name="avs", bufs=2) as avsp,
            tc.tile_pool(name="rp", bufs=2) as rp,
            tc.tile_pool(name="ob", bufs=3) as obp,
            tc.tile_pool(name="qkv_ps", bufs=2, space="PSUM") as qkv_ps,
            tc.tile_pool(name="sim_ps", bufs=2, space="PSUM") as sim_ps,
            tc.tile_pool(name="av_ps", bufs=2, space="PSUM") as av_ps,
        ):
            # ---------------- DMA staging ----------------
            xraws = []
            for bb in range(PB):
                xr = xp.tile([128, CT, N], F32, tag="xraw")
                xraws.append(xr)

            wqkv = const.tile([128, CT, 3 * C], BF16, tag="wqkv")
            wo = const.tile([128, CT, C], BF16, tag="wo")
            g1 = const.tile([128, CT], F32, tag="g1")
            ones1 = const.tile([128, 64], F32R, tag="ones1")
            ones128 = const.tile([128, 128], BF16, tag="ones128")
            # kT packed per head-pair: rows 0:64 = even head (d), 64:128 = odd
            kTp = const.tile([128, NPAIR, 1028], BF16, tag="kTp")
            vextA = const.tile([128, 8, VW], BF16, tag="vextA")
            vextB = const.tile([128, 8, VW], BF16, tag="vextB")
            vmem = const.tile([128, 2, VW], BF16, tag="vmem")
            gsb = const.tile([128, CT], F32, tag="gsb")
            vexts = [vextA, vextB]

            # x0 split across all four DMA queues so norm-0 starts asap
            xq = [nc.sync, nc.scalar, nc.gpsimd, nc.sync]
            for t in range(CT):
                xq[t].dma_start(out=xraws[0][:, t, :], in_=x_ext[0, t * 128:(t + 1) * 128, :])
            nc.scalar.dma_start(out=gsb, in_=gammat_ext[:, :])
            for t in range(CT):
                wq = wqs.tile([128, 3 * C], F32, tag="wq")
                (nc.sync if t < 2 else nc.scalar).dma_start(
                    out=wq, in_=wqkvt_ext[t * 128:(t + 1) * 128, :])
                nc.vector.tensor_copy(out=wqkv[:, t, :], in_=wq)
            ws = wstage.tile([128, NPAIR * NMEM + 2 * VW], F32, tag="ws")
            nc.gpsimd.dma_start(out=ws[:, 0:NPAIR * NMEM],
                                in_=memk_ext[:, :, :].rearrange("p g c -> p (g c)"))
            nc.gpsimd.dma_start(out=ws[:, NPAIR * NMEM:NPAIR * NMEM + 2 * VW],
                                in_=memv_ext[:, :, :].rearrange("p g c -> p (g c)"))
            for t in range(CT):
                (nc.gpsimd if t < 2 else nc.scalar).dma_start(
                    out=xraws[1][:, t, :], in_=x_ext[1, t * 128:(t + 1) * 128, :])
            # out-proj weights are only needed late; keep them last on sync
            wostages = []
            for t in range(CT):
                wst = wstage.tile([128, C], F32, tag="wst" + str(t))
                nc.sync.dma_start(out=wst, in_=wot_ext[t * 128:(t + 1) * 128, :])
                wostages.append(wst)

            def wo_cast_unit(t):
                return lambda: nc.vector.tensor_copy(out=wo[:, t, :], in_=wostages[t])

            nc.scalar.activation(out=g1, in_=gsb, func=AF.Copy, bias=1.0)
            nc.vector.memset(ones128, 1.0)
            nc.vector.memset(ones1.bitcast(F32), 1.0)
            nc.vector.tensor_copy(
                out=kTp[:, :, 1024:1028],
                in_=ws[:, 0:NPAIR * NMEM].rearrange("p (g c) -> p g c", c=NMEM))
            nc.vector.tensor_copy(
                out=vmem,
                in_=ws[:, NPAIR * NMEM:NPAIR * NMEM + 2 * VW].rearrange("p (g c) -> p g c", c=VW))
            for v in vexts:
                oc = v[:, :, :].rearrange("p j (h c) -> p j h c", c=DH + 1)[:, :, :, DH:DH + 1]
                nc.gpsimd.memset(oc, 1.0)

            # ---------------- pipeline stages ----------------
            def norm(bb):
                """x -> xn = x * (gamma+1) / rms(x)  (bf16)."""
                xraw = xraws[bb]
                xsq = data.tile([128, CT, N], BF16, tag="xsq")
                for t in range(CT):
                    nc.scalar.activation(out=xsq[:, t, :], in_=xraw[:, t, :], func=AF.Square)
                ss = sim_ps.tile([128, N], F32, tag="sim")
                for h2 in range(2):
                    for t in range(CT):
                        nc.tensor.matmul(ss[:, h2 * 512:(h2 + 1) * 512], ones128,
                                         xsq[:, t, h2 * 512:(h2 + 1) * 512],
                                         start=(t == 0), stop=(t == CT - 1))
                sroot = data.tile([128, N], F32, tag="sroot")
                nc.scalar.activation(out=sroot, in_=ss, func=AF.Sqrt, scale=1.0 / C)
                snorm = data.tile([128, N], F32, tag="snorm")
                nc.vector.reciprocal_approx_fast(out=snorm, in_=sroot)
                xn = data.tile([128, CT, N], BF16, tag="xn" + str(bb))
                for t in range(CT):
                    nc.vector.scalar_tensor_tensor(
                        out=xn[:, t, :], in0=xraw[:, t, :], scalar=g1[:, t:t + 1],
                        in1=snorm, op0=mybir.AluOpType.mult, op1=mybir.AluOpType.mult)
                return xn

            def qkproj(xn, qT, mcs):
                for mc in mcs:
                    for h2 in range(2):
                        ps = qkv_ps.tile([128, 512], F32, tag="q")
                        for t in range(CT):
                            nc.tensor.matmul(ps, wqkv[:, t, mc * 128:(mc + 1) * 128],
                                             xn[:, t, h2 * 512:(h2 + 1) * 512],
                                             start=(t == 0), stop=(t == CT - 1))
                        if mc < 4:
                            nc.vector.tensor_copy(out=qT[:, mc, h2 * 512:(h2 + 1) * 512], in_=ps)
                        else:
                            nc.vector.tensor_copy(
                                out=kTp[:, mc - 4, h2 * 512:(h2 + 1) * 512], in_=ps)

            def qkproj_units(xn, qT, mc):
                state = {}
                units = []
                for h2 in range(2):
                    def u1(h2=h2):
                        ps = qkv_ps.tile([128, 512], F32, tag="q")
                        state[h2] = ps
                        for t in (0, 1):
                            nc.tensor.matmul(ps, wqkv[:, t, mc * 128:(mc + 1) * 128],
                                             xn[:, t, h2 * 512:(h2 + 1) * 512],
                                             start=(t == 0), stop=False)

                    def u2(h2=h2):
                        ps = state[h2]
                        for t in (2, 3):
                            nc.tensor.matmul(ps, wqkv[:, t, mc * 128:(mc + 1) * 128],
                                             xn[:, t, h2 * 512:(h2 + 1) * 512],
                                             start=False, stop=(t == CT - 1))
                        if mc < 4:
                            nc.vector.tensor_copy(out=qT[:, mc, h2 * 512:(h2 + 1) * 512], in_=ps)
                        else:
                            nc.vector.tensor_copy(
                                out=kTp[:, mc - 4, h2 * 512:(h2 + 1) * 512], in_=ps)
                    units.append(u1)
                    units.append(u2)
                return units

            def vproj(xn, vext, ics):
                for ic in ics:
                    ps = qkv_ps.tile([128, 512], F32, tag="q")
                    for t in range(CT):
                        nc.tensor.matmul(ps, xn[:, t, ic * 128:(ic + 1) * 128],
                                         wqkv[:, t, 2 * C:3 * C],
                                         start=(t == 0), stop=(t == CT - 1))
                    ps_h = ps[:, :].rearrange("p (h c) -> p h c", c=DH)
                    vdst = vext[:, ic, :].rearrange("p (h c) -> p h c", c=DH + 1)[:, :, 0:DH]
                    nc.vector.tensor_copy(out=vdst, in_=ps_h)

            def vproj_units(xn, vext, ic):
                state = {}

                def u1():
                    ps = qkv_ps.tile([128, 512], F32, tag="q")
                    state[0] = ps
                    for t in (0, 1):
                        nc.tensor.matmul(ps, xn[:, t, ic * 128:(ic + 1) * 128],
                                         wqkv[:, t, 2 * C:3 * C],
                                         start=(t == 0), stop=False)

                def u2():
                    ps = state[0]
                    for t in (2, 3):
                        nc.tensor.matmul(ps, xn[:, t, ic * 128:(ic + 1) * 128],
                                         wqkv[:, t, 2 * C:3 * C],
                                         start=False, stop=(t == CT - 1))
                    ps_h = ps[:, :].rearrange("p (h c) -> p h c", c=DH)
                    vdst = vext[:, ic, :].rearrange("p (h c) -> p h c", c=DH + 1)[:, :, 0:DH]
                    nc.vector.tensor_copy(out=vdst, in_=ps_h)
                return [u1, u2]

            def proj(attn, bb, mcs=None, h2s=(0, 1)):
                for mc in (range(CT) if mcs is None else mcs):
                    for h2 in h2s:
                        ps = qkv_ps.tile([128, 512], F32, tag="q")
                        for t in range(CT):
                            nc.tensor.matmul(ps, wo[:, t, mc * 128:(mc + 1) * 128],
                                             attn[:, t, h2 * 512:(h2 + 1) * 512],
                                             start=(t == 0), stop=(t == CT - 1))
                        ob = obp.tile([128, 512], F32, tag="ob")
                        nc.vector.tensor_copy(out=ob, in_=ps)
                        nc.sync.dma_start(
                            out=out_ext[bb, mc * 128:(mc + 1) * 128, h2 * 512:(h2 + 1) * 512],
                            in_=ob)

            def proj_units(attn, bb, mc, h2):
                state = {}

                def u1():
                    ps = qkv_ps.tile([128, 512], F32, tag="q")
                    state[0] = ps
                    for t in (0, 1):
                        nc.tensor.matmul(ps, wo[:, t, mc * 128:(mc + 1) * 128],
                                         attn[:, t, h2 * 512:(h2 + 1) * 512],
                                         start=(t == 0), stop=False)

                def u2():
                    ps = state[0]
                    for t in (2, 3):
                        nc.tensor.matmul(ps, wo[:, t, mc * 128:(mc + 1) * 128],
                                         attn[:, t, h2 * 512:(h2 + 1) * 512],
                                         start=False, stop=(t == CT - 1))
                    ob = obp.tile([128, 512], F32, tag="ob")
                    nc.vector.tensor_copy(out=ob, in_=ps)
                    nc.sync.dma_start(
                        out=out_ext[bb, mc * 128:(mc + 1) * 128, h2 * 512:(h2 + 1) * 512],
                        in_=ob)
                return [u1, u2]

            def memsim(qT, pmems, g):
                st = sim_ps.tile([128, N], F32, tag="sim")
                for h4 in range(4):
                    h = 4 * g + h4
                    p, hh = h // 2, h % 2
                    for h2 in range(2):
                        nc.tensor.matmul(
                            st[32 * h4:32 * h4 + NMEM, h2 * 512:(h2 + 1) * 512],
                            kTp[64 * hh:64 * hh + 64, p, 1024:1028],
                            qT[64 * hh:64 * hh + 64, p, h2 * 512:(h2 + 1) * 512],
                            start=True, stop=True, tile_position=(64 * hh, 32 * h4))
                pmt = pm.tile([128, N], BF16, tag="pm")
                nc.scalar.activation(out=pmt, in_=st, func=AF.Exp, scale=QSC)
                pmems[g] = pmt

            def memsim_unit(qT, pmems, g):
                return lambda: memsim(qT, pmems, g)

            def attention(qT, vext, attn, pmems, fill):
                """All 8 heads as one flat stream of 68 rounds."""
                state = {}
                avbs = {}
                pts = [None] * 64

                def sim_round(r):
                    s, jc = r // 8, r % 8
                    p, h2 = s // 2, s % 2
                    st = sim_ps.tile([128, N], F32, tag="sim")
                    for hh in range(2):
                        nc.tensor.matmul(
                            st[:, hh * 512:(hh + 1) * 512],
                            kTp[64 * hh:64 * hh + 64, p, jc * 128:(jc + 1) * 128],
                            qT[64 * hh:64 * hh + 64, p, h2 * 512:(h2 + 1) * 512],
                            start=True, stop=True)
                    pt = pp.tile([128, N], BF16, tag="p")
                    nc.scalar.activation(out=pt, in_=st, func=AF.Exp, scale=QSC)
                    pts[r] = pt

                def av_round(r):
                    s, jc = r // 8, r % 8
                    p, h2 = s // 2, s % 2
                    if jc == 0:
                        avA = av_ps.tile([65, 512], F32, tag="av")
                        avB = av_ps.tile([65, 512], F32, tag="av")
                        state[s] = (avA, avB)
                    avt = state[s]
                    for hh in range(2):
                        h = 2 * p + hh
                        nc.tensor.matmul(
                            avt[hh], vext[:, jc, h * (DH + 1):(h + 1) * (DH + 1)],
                            pts[r][:, hh * 512:(hh + 1) * 512],
                            start=(jc == 0), stop=False)

                def epilogue_a(s):
                    """mem-kv contribution + PSUM evacuation (frees av tiles)."""
                    p, h2 = s // 2, s % 2
                    avt = state.pop(s)
                    for hh in range(2):
                        h = 2 * p + hh
                        g, r0 = h // 4, 32 * (h % 4)
                        nc.tensor.matmul(
                            avt[hh],
                            vmem[r0:r0 + NMEM, g, (h % 4) * (DH + 1):(h % 4 + 1) * (DH + 1)],
                            pmems[g][r0:r0 + NMEM, h2 * 512:(h2 + 1) * 512],
                            start=False, stop=True, tile_position=(r0, 0))
                    pair = []
                    for hh in range(2):
                        avb = avsp.tile([65, 512], F32R, tag="avs")
                        with tc.high_priority(offset=64):
                            nc.vector.tensor_copy(out=avb, in_=avt[hh])
                        pair.append(avb)
                    avbs[s] = pair

                def epilogue_b(s):
                    """normalize: attn = av[0:64] / av[64], two rounds later
                    so bc never heads the PE queue waiting for the copy."""
                    p, h2 = s // 2, s % 2
                    for hh in range(2):
                        avb = avbs[s][hh]
                        bc = qkv_ps.tile([64, 512], F32, tag="q")
                        nc.tensor.matmul(bc, ones1[64:65, :], avb[64:65, :], start=True, stop=True)
                        rcp = rp.tile([64, 512], F32, tag="rcp")
                        nc.vector.reciprocal_approx_fast(out=rcp, in_=bc)
                        nc.vector.tensor_mul(
                            out=attn[64 * hh:64 * hh + 64, p, h2 * 512:(h2 + 1) * 512],
                            in0=avb[0:64, :].bitcast(F32), in1=rcp)

                for r in range(68):
                    if r < 64:
                        sim_round(r)
                    if 2 <= r:
                        ar = r - 2
                        if ar < 64:
                            av_round(ar)
                        if ar % 8 == 7 and ar // 8 < 8:
                            epilogue_a(ar // 8)
                    if 4 <= r:
                        br = r - 4
                        if br % 8 == 7 and br // 8 < 8:
                            epilogue_b(br // 8)
                    for u in (fill[r] if r < len(fill) else []):
                        u()

            # ---------------- schedule ----------------
            # Prologue: only what batch-0's first sweeps need.
            xn0 = norm(0)
            qT0 = qp.tile([128, CT, N], BF16, tag="qT")
            qT1 = qp.tile([128, CT, N], BF16, tag="qT")
            qkproj(xn0, qT0, [0, 1, 4])      # q pairs 0,1 + k pair 0
            pmem0 = [None, None]
            memsim(qT0, pmem0, 0)            # heads 0-3 (needs q0, q1)
            xn1 = norm(1)

            attn0 = atp.tile([128, CT, N], BF16, tag="attn")
            attn1 = atp.tile([128, CT, N], BF16, tag="attn")
            pmem1 = [None, None]

            def place(fill, r0, units, per_round=1):
                r, i = r0, 0
                while i < len(units):
                    for _ in range(per_round):
                        if i < len(units):
                            fill[r].append(units[i])
                            i += 1
                    r += 1

            # batch-0 attention fill: own v4-v7, q2-q3, mem group 1,
            # k1-k3, then batch-1's q/k0/v and its mem sims.
            # kTp pair i is last read by batch-0 at round 8*(2i+1)+7.
            f0 = [[] for _ in range(68)]
            place(f0, 0, [u for ic in range(8) for u in vproj_units(xn0, vexts[0], ic)],
                  per_round=2)                             # v0-v7 by round 7
            place(f0, 8, qkproj_units(xn0, qT0, 5))        # k1, read from rnd 16
            place(f0, 12, qkproj_units(xn0, qT0, 6))       # k2, read from rnd 32
            place(f0, 16, qkproj_units(xn0, qT0, 7))       # k3, read from rnd 48
            place(f0, 20, qkproj_units(xn0, qT0, 2))       # q pair 2
            place(f0, 24, qkproj_units(xn0, qT0, 3))       # q pair 3
            place(f0, 29, [memsim_unit(qT0, pmem0, 1)])    # heads 4-7, by rnd 41
            place(f0, 30, qkproj_units(xn1, qT1, 0))
            place(f0, 34, qkproj_units(xn1, qT1, 1))
            place(f0, 38, qkproj_units(xn1, qT1, 4))       # batch-1 k0 (safe: >15)
            place(f0, 42, qkproj_units(xn1, qT1, 2))
            place(f0, 46, qkproj_units(xn1, qT1, 3))
            place(f0, 50, [u for ic in range(8) for u in vproj_units(xn1, vexts[1], ic)],
                  per_round=2)                             # rounds 50-57
            place(f0, 58, [wo_cast_unit(t) for t in range(CT)])
            attention(qT0, vexts[0], attn0, pmem0, f0)

            # batch-1 attention fill: own k1-k3, then batch-0 out-proj.
            f1 = [[] for _ in range(68)]
            place(f1, 0, [memsim_unit(qT1, pmem1, 0), memsim_unit(qT1, pmem1, 1)])
            place(f1, 2, qkproj_units(xn1, qT1, 5))
            place(f1, 6, qkproj_units(xn1, qT1, 6))
            place(f1, 10, qkproj_units(xn1, qT1, 7))
            place(f1, 14, [u for mc in range(CT) for h2 in range(2)
                           for u in proj_units(attn0, 0, mc, h2)])
            attention(qT1, vexts[1], attn1, pmem1, f1)
            proj(attn1, 1)
    nc.compile()
    return nc


_NC_CACHE = []


def kernel(x, gamma, mem_kv, w_qkv, w_out, _trace=False):
    x = np.asarray(x, dtype=np.float32)
    gamma = np.asarray(gamma, dtype=np.float32)
    mem_kv = np.asarray(mem_kv, dtype=np.float32)
    w_qkv = np.asarray(w_qkv, dtype=np.float32)
    w_out = np.asarray(w_out, dtype=np.float32)

    b, c, hh, ww = x.shape
    n = hh * ww
    xs = x.reshape(b, c, n)

    wqkvt = np.ascontiguousarray(w_qkv.T)          # [c, 3c]
    wot = np.ascontiguousarray(w_out.T)            # [c, c]
    gammat = np.ascontiguousarray(gamma.reshape(CT, 128).T)  # [128, CT]

    memk = np.zeros((128, NPAIR, NMEM), np.float32)
    memv = np.zeros((128, 2, VW), np.float32)
    for h in range(HEADS):
        p, hh_ = h // 2, h % 2
        memk[64 * hh_:64 * hh_ + DH, p, 0:NMEM] = mem_kv[0, h].T  # [dh, nmem]
        g, r1, c0 = h // 4, 32 * (h % 4), (h % 4) * (DH + 1)
        memv[r1:r1 + NMEM, g, c0:c0 + DH] = mem_kv[1, h]
        memv[r1:r1 + NMEM, g, c0 + DH] = 1.0

    if not _NC_CACHE:
        _NC_CACHE.append(_build())
    nc = _NC_CACHE[0]

    in_maps = []
    for core in range(NCORES):
        in_maps.append({
            "x": np.ascontiguousarray(xs[core * PB:(core + 1) * PB]),
            "wqkvt": wqkvt,
            "wot": wot,
            "gammat": gammat,
            "memk": memk,
            "memv": memv,
        })
    res = run_bass_kernel_spmd(nc, in_maps, core_ids=list(range(NCORES)), trace=_trace)
    out = np.concatenate([res.results[core]["out"] for core in range(NCORES)], axis=0)
    kernel.last_result = res
    return out.reshape(b, c, hh, ww)


# revision 24
# speedup vs baseline: 1.2398x; 1.0015x over previous
"""Trainium2 Bass kernel for nn_Attention_7945689497706.

v5 structure:
- K=64 row-tiled sim matmul pairs (both heads of a pair concurrently on
  PE row groups 0:64 / 64:128 — no zero padding).
- Attention as one flat 68-round stream per batch: round r issues the
  sim pair of (sweep r//8, jc r%8), the av matmuls of round r-2, and the
  sweep epilogues lagged so no PE instruction heads the queue waiting.
- Weights kept in f32r, DMA'd straight into SBUF: gamma folds into xn
  (scalar_tensor_tensor), the q-scale dh^-0.5 folds into exp's scale.
- Each batch self-fills its q/k/v projections and the other batch's
  work into its own exp-bound attention bubbles as 2-MM units.
"""

import numpy as np

import concourse.bass as bass
import concourse.mybir as mybir
import concourse.tile as tile
from concourse import bacc
from concourse.bass_utils import run_bass_kernel_spmd

F32 = mybir.dt.float32
F32R = mybir.dt.float32r
BF16 = mybir.dt.bfloat16
AF = mybir.ActivationFunctionType

NCORES = 8
B = 16
C = 512
N = 1024          # pixels = 32*32
HEADS = 8
DH = 64
NMEM = 4
PB = B // NCORES  # batch elements per core
CT = C // 128     # channel partition-tiles
NPAIR = HEADS // 2
VW = HEADS * (DH + 1)  # vext width: per head [v | ones] = 65
QSC = DH ** -0.5


def _build():
    nc = bacc.Bacc()
    x_ext = nc.declare_dram_parameter("x", [PB, C, N], F32, isOutput=False)
    wqkvt_ext = nc.declare_dram_parameter("wqkvt", [C, 3 * C], F32, isOutput=False)
    wot_ext = nc.declare_dram_parameter("wot", [C, C], F32, isOutput=False)
    gammat_ext = nc.declare_dram_parameter("gammat", [128, CT], F32, isOutput=False)
    memk_ext = nc.declare_dram_parameter("memk", [128, NPAIR, NMEM], F32, isOutput=False)
    memv_ext = nc.declare_dram_parameter("memv", [128, 2, VW], F32, isOutput=False)
    out_ext = nc.declare_dram_parameter("out", [PB, C, N], F32, isOutput=True)

    with tile.TileContext(nc) as tc:
        with (
            tc.tile_pool(name="const", bufs=1) as const,
            tc.tile_pool(name="wstage", bufs=1) as wstage,
            tc.tile_pool(name="wqs", bufs=2) as wqs,
            tc.tile_pool(name="xp", bufs=2) as xp,
            tc.tile_pool(name="data", bufs=1) as data,
            tc.tile_pool(name="atp", bufs=2) as atp,
            tc.tile_pool(name="qp", bufs=2) as qp,
            tc.tile_pool(name="pp", bufs=4) as pp,
            tc.tile_pool(name="pm", bufs=2) as pm,
            tc.tile_pool(name="avs", bufs=2) as avsp,
            tc.tile_pool(name="rp", bufs=2) as rp,
            tc.tile_pool(name="ob", bufs=3) as obp,
            tc.tile_pool(name="qkv_ps", bufs=2, space="PSUM") as qkv_ps,
            tc.tile_pool(name="sim_ps", bufs=2, space="PSUM") as sim_ps,
            tc.tile_pool(name="av_ps", bufs=2, space="PSUM") as av_ps,
        ):
            # ---------------- DMA staging ----------------
            xraws = []
            for bb in range(PB):
                xr = xp.tile([128, CT, N], F32, tag="xraw")
                xraws.append(xr)

            wqkv = const.tile([128, CT, 3 * C], BF16, tag="wqkv")
            wo = const.tile([128, CT, C], BF16, tag="wo")
            g1 = const.tile([128, CT], F32, tag="g1")
            ones1 = const.tile([128, 64], F32R, tag="ones1")
            ones128 = const.tile([128, 128], BF16, tag="ones128")
            # kT packed per head-pair: rows 0:64 = even head (d), 64:128 = odd
            kTp = const.tile([128, NPAIR, 1028], BF16, tag="kTp")
            vextA = const.tile([128, 8, VW], BF16, tag="vextA")
            vextB = const.tile([128, 8, VW], BF16, tag="vextB")
            vmem = const.tile([128, 2, VW], BF16, tag="vmem")
            gsb = const.tile([128, CT], F32, tag="gsb")
            vexts = [vextA, vextB]

            # x0 split across all four DMA queues so norm-0 starts asap
            xq = [nc.sync, nc.scalar, nc.gpsimd, nc.sync]
            for t in range(CT):
                xq[t].dma_start(out=xraws[0][:, t, :], in_=x_ext[0, t * 128:(t + 1) * 128, :])
            nc.scalar.dma_start(out=gsb, in_=gammat_ext[:, :])
            wqstages = {}
            for t in range(CT):
                wq = wqs.tile([128, 3 * C], F32, tag="wq" + str(t // 2))
                (nc.sync if t < 2 else nc.scalar).dma_start(
                    out=wq, in_=wqkvt_ext[t * 128:(t + 1) * 128, :])
                wqstages[t] = wq
                if t < 2:
                    nc.vector.tensor_copy(out=wqkv[:, t, :], in_=wq)
            ws = wstage.tile([128, NPAIR * NMEM + 2 * VW], F32, tag="ws")
            nc.gpsimd.dma_start(out=ws[:, 0:NPAIR * NMEM],
                                in_=memk_ext[:, :, :].rearrange("p g c -> p (g c)"))
            nc.gpsimd.dma_start(out=ws[:, NPAIR * NMEM:NPAIR * NMEM + 2 * VW],
                                in_=memv_ext[:, :, :].rearrange("p g c -> p (g c)"))
            for t in range(CT):
                (nc.gpsimd if t < 2 else nc.scalar).dma_start(
                    out=xraws[1][:, t, :], in_=x_ext[1, t * 128:(t + 1) * 128, :])
            # out-proj weights are only needed late; keep them last on sync
            wostages = []
            for t in range(CT):
                wst = wstage.tile([128, C], F32, tag="wst" + str(t))
                nc.sync.dma_start(out=wst, in_=wot_ext[t * 128:(t + 1) * 128, :])
                wostages.append(wst)

            def wo_cast_unit(t):
                return lambda: nc.vector.tensor_copy(out=wo[:, t, :], in_=wostages[t])

            nc.scalar.activation(out=g1, in_=gsb, func=AF.Copy, bias=1.0)
            nc.vector.memset(ones128, 1.0)
            nc.vector.memset(ones1.bitcast(F32), 1.0)
            nc.vector.tensor_copy(
                out=kTp[:, :, 1024:1028],
                in_=ws[:, 0:NPAIR * NMEM].rearrange("p (g c) -> p g c", c=NMEM))
            nc.vector.tensor_copy(
                out=vmem,
                in_=ws[:, NPAIR * NMEM:NPAIR * NMEM + 2 * VW].rearrange("p (g c) -> p g c", c=VW))
            for v in vexts:
                oc = v[:, :, :].rearrange("p j (h c) -> p j h c", c=DH + 1)[:, :, :, DH:DH + 1]
                nc.gpsimd.memset(oc, 1.0)

            # ---------------- pipeline stages ----------------
            def norm(bb):
                """x -> xn = x * (gamma+1) / rms(x)  (bf16)."""
                xraw = xraws[bb]
                xsq = data.tile([128, CT, N], BF16, tag="xsq")
                for t in range(CT):
                    nc.scalar.activation(out=xsq[:, t, :], in_=xraw[:, t, :], func=AF.Square)
                ss = sim_ps.tile([128, N], F32, tag="sim")
                for h2 in range(2):
                    for t in range(CT):
                        nc.tensor.matmul(ss[:, h2 * 512:(h2 + 1) * 512], ones128,
                                         xsq[:, t, h2 * 512:(h2 + 1) * 512],
                                         start=(t == 0), stop=(t == CT - 1))
                sroot = data.tile([128, N], F32, tag="sroot")
                nc.scalar.activation(out=sroot, in_=ss, func=AF.Sqrt, scale=1.0 / C)
                snorm = data.tile([128, N], F32, tag="snorm")
                nc.vector.reciprocal_approx_fast(out=snorm, in_=sroot)
                xn = data.tile([128, CT, N], BF16, tag="xn" + str(bb))

                def xn_unit(t):
                    def u():
                        nc.vector.scalar_tensor_tensor(
                            out=xn[:, t, :], in0=xraw[:, t, :], scalar=g1[:, t:t + 1],
                            in1=snorm, op0=mybir.AluOpType.mult, op1=mybir.AluOpType.mult)
                    return u
                return xn, xn_unit

            def qkproj(xn, qT, mcs):
                for mc in mcs:
                    for h2 in range(2):
                        ps = qkv_ps.tile([128, 512], F32, tag="q")
                        for t in range(CT):
                            nc.tensor.matmul(ps, wqkv[:, t, mc * 128:(mc + 1) * 128],
                                             xn[:, t, h2 * 512:(h2 + 1) * 512],
                                             start=(t == 0), stop=(t == CT - 1))
                        if mc < 4:
                            nc.vector.tensor_copy(out=qT[:, mc, h2 * 512:(h2 + 1) * 512], in_=ps)
                        else:
                            nc.vector.tensor_copy(
                                out=kTp[:, mc - 4, h2 * 512:(h2 + 1) * 512], in_=ps)

            def qkproj_units(xn, qT, mc):
                state = {}
                units = []
                for h2 in range(2):
                    def u1(h2=h2):
                        ps = qkv_ps.tile([128, 512], F32, tag="q")
                        state[h2] = ps
                        for t in (0, 1):
                            nc.tensor.matmul(ps, wqkv[:, t, mc * 128:(mc + 1) * 128],
                                             xn[:, t, h2 * 512:(h2 + 1) * 512],
                                             start=(t == 0), stop=False)

                    def u2(h2=h2):
                        ps = state[h2]
                        for t in (2, 3):
                            nc.tensor.matmul(ps, wqkv[:, t, mc * 128:(mc + 1) * 128],
                                             xn[:, t, h2 * 512:(h2 + 1) * 512],
                                             start=False, stop=(t == CT - 1))
                        if mc < 4:
                            nc.vector.tensor_copy(out=qT[:, mc, h2 * 512:(h2 + 1) * 512], in_=ps)
                        else:
                            nc.vector.tensor_copy(
                                out=kTp[:, mc - 4, h2 * 512:(h2 + 1) * 512], in_=ps)
                    units.append(u1)
                    units.append(u2)
                return units

            def vproj(xn, vext, ics):
                for ic in ics:
                    ps = qkv_ps.tile([128, 512], F32, tag="q")
                    for t in range(CT):
                        nc.tensor.matmul(ps, xn[:, t, ic * 128:(ic + 1) * 128],
                                         wqkv[:, t, 2 * C:3 * C],
                                         start=(t == 0), stop=(t == CT - 1))
                    ps_h = ps[:, :].rearrange("p (h c) -> p h c", c=DH)
                    vdst = vext[:, ic, :].rearrange("p (h c) -> p h c", c=DH + 1)[:, :, 0:DH]
                    nc.vector.tensor_copy(out=vdst, in_=ps_h)

            def vproj_units(xn, vext, ic):
                state = {}

                def u1():
                    ps = qkv_ps.tile([128, 512], F32, tag="q")
                    state[0] = ps
                    for t in (0, 1):
                        nc.tensor.matmul(ps, xn[:, t, ic * 128:(ic + 1) * 128],
                                         wqkv[:, t, 2 * C:3 * C],
                                         start=(t == 0), stop=False)

                def u2():
                    ps = state[0]
                    for t in (2, 3):
                        nc.tensor.matmul(ps, xn[:, t, ic * 128:(ic + 1) * 128],
                                         wqkv[:, t, 2 * C:3 * C],
                                         start=False, stop=(t == CT - 1))
                    ps_h = ps[:, :].rearrange("p (h c) -> p h c", c=DH)
                    vdst = vext[:, ic, :].rearrange("p (h c) -> p h c", c=DH + 1)[:, :, 0:DH]
                    nc.vector.tensor_copy(out=vdst, in_=ps_h)
                return [u1, u2]

            def proj(attn, bb, mcs=None, h2s=(0, 1)):
                for mc in (range(CT) if mcs is None else mcs):
                    for h2 in h2s:
                        ps = qkv_ps.tile([128, 512], F32, tag="q")
                        for t in range(CT):
                            nc.tensor.matmul(ps, wo[:, t, mc * 128:(mc + 1) * 128],
                                             attn[:, t, h2 * 512:(h2 + 1) * 512],
                                             start=(t == 0), stop=(t == CT - 1))
                        ob = obp.tile([128, 512], F32, tag="ob")
                        nc.vector.tensor_copy(out=ob, in_=ps)
                        nc.sync.dma_start(
                            out=out_ext[bb, mc * 128:(mc + 1) * 128, h2 * 512:(h2 + 1) * 512],
                            in_=ob)

            def proj_units(attn, bb, mc, h2):
                state = {}

                def u1():
                    ps = qkv_ps.tile([128, 512], F32, tag="q")
                    state[0] = ps
                    for t in (0, 1):
                        nc.tensor.matmul(ps, wo[:, t, mc * 128:(mc + 1) * 128],
                                         attn[:, t, h2 * 512:(h2 + 1) * 512],
                                         start=(t == 0), stop=False)

                def u2():
                    ps = state[0]
                    for t in (2, 3):
                        nc.tensor.matmul(ps, wo[:, t, mc * 128:(mc + 1) * 128],
                                         attn[:, t, h2 * 512:(h2 + 1) * 512],
                                         start=False, stop=(t == CT - 1))
                    ob = obp.tile([128, 512], F32, tag="ob")
                    nc.vector.tensor_copy(out=ob, in_=ps)
                    nc.sync.dma_start(
                        out=out_ext[bb, mc * 128:(mc + 1) * 128, h2 * 512:(h2 + 1) * 512],
                        in_=ob)
                return [u1, u2]

            def memsim(qT, pmems, g):
                st = sim_ps.tile([128, N], F32, tag="sim")
                for h4 in range(4):
                    h = 4 * g + h4
                    p, hh = h // 2, h % 2
                    for h2 in range(2):
                        nc.tensor.matmul(
                            st[32 * h4:32 * h4 + NMEM, h2 * 512:(h2 + 1) * 512],
                            kTp[64 * hh:64 * hh + 64, p, 1024:1028],
                            qT[64 * hh:64 * hh + 64, p, h2 * 512:(h2 + 1) * 512],
                            start=True, stop=True, tile_position=(64 * hh, 32 * h4))
                pmt = pm.tile([128, N], BF16, tag="pm")
                nc.scalar.activation(out=pmt, in_=st, func=AF.Exp, scale=QSC)
                pmems[g] = pmt

            def memsim_unit(qT, pmems, g):
                return lambda: memsim(qT, pmems, g)

            def attention(qT, vext, attn, pmems, fill):
                """All 8 heads as one flat stream of 68 rounds."""
                state = {}
                avbs = {}
                pts = [None] * 64

                def sim_round(r):
                    s, jc = r // 8, r % 8
                    p, h2 = s // 2, s % 2
                    st = sim_ps.tile([128, N], F32, tag="sim")
                    for hh in range(2):
                        nc.tensor.matmul(
                            st[:, hh * 512:(hh + 1) * 512],
                            kTp[64 * hh:64 * hh + 64, p, jc * 128:(jc + 1) * 128],
                            qT[64 * hh:64 * hh + 64, p, h2 * 512:(h2 + 1) * 512],
                            start=True, stop=True)
                    pt = pp.tile([128, N], BF16, tag="p")
                    nc.scalar.activation(out=pt, in_=st, func=AF.Exp, scale=QSC)
                    pts[r] = pt

                def av_round(r):
                    s, jc = r // 8, r % 8
                    p, h2 = s // 2, s % 2
                    if jc == 0:
                        avA = av_ps.tile([65, 512], F32, tag="av")
                        avB = av_ps.tile([65, 512], F32, tag="av")
                        state[s] = (avA, avB)
                    avt = state[s]
                    for hh in range(2):
                        h = 2 * p + hh
                        nc.tensor.matmul(
                            avt[hh], vext[:, jc, h * (DH + 1):(h + 1) * (DH + 1)],
                            pts[r][:, hh * 512:(hh + 1) * 512],
                            start=(jc == 0), stop=False)

                def epilogue_a(s):
                    """mem-kv contribution + PSUM evacuation (frees av tiles)."""
                    p, h2 = s // 2, s % 2
                    avt = state.pop(s)
                    for hh in range(2):
                        h = 2 * p + hh
                        g, r0 = h // 4, 32 * (h % 4)
                        nc.tensor.matmul(
                            avt[hh],
                            vmem[r0:r0 + NMEM, g, (h % 4) * (DH + 1):(h % 4 + 1) * (DH + 1)],
                            pmems[g][r0:r0 + NMEM, h2 * 512:(h2 + 1) * 512],
                            start=False, stop=True, tile_position=(r0, 0))
                    pair = []
                    for hh in range(2):
                        avb = avsp.tile([65, 512], F32R, tag="avs")
                        with tc.high_priority(offset=64):
                            nc.vector.tensor_copy(out=avb, in_=avt[hh])
                        pair.append(avb)
                    avbs[s] = pair

                def epilogue_b(s):
                    """normalize: attn = av[0:64] / av[64], two rounds later
                    so bc never heads the PE queue waiting for the copy."""
                    p, h2 = s // 2, s % 2
                    for hh in range(2):
                        avb = avbs[s][hh]
                        bc = qkv_ps.tile([64, 512], F32, tag="q")
                        nc.tensor.matmul(bc, ones1[64:65, :], avb[64:65, :], start=True, stop=True)
                        rcp = rp.tile([64, 512], F32, tag="rcp")
                        nc.vector.reciprocal_approx_fast(out=rcp, in_=bc)
                        nc.vector.tensor_mul(
                            out=attn[64 * hh:64 * hh + 64, p, h2 * 512:(h2 + 1) * 512],
                            in0=avb[0:64, :].bitcast(F32), in1=rcp)

                for r in range(68):
                    if r < 64:
                        sim_round(r)
                    if 2 <= r:
                        ar = r - 2
                        if ar < 64:
                            av_round(ar)
                        if ar % 8 == 7 and ar // 8 < 8:
                            epilogue_a(ar // 8)
                    if 4 <= r:
                        br = r - 4
                        if br % 8 == 7 and br // 8 < 8:
                            epilogue_b(br // 8)
                    for u in (fill[r] if r < len(fill) else []):
                        u()

            # ---------------- schedule ----------------
            xn0, xn0_units = norm(0)
            for t in range(CT):
                xn0_units(t)()
            for t in (2, 3):
                nc.vector.tensor_copy(out=wqkv[:, t, :], in_=wqstages[t])
            qT0 = qp.tile([128, CT, N], BF16, tag="qT")
            qT1 = qp.tile([128, CT, N], BF16, tag="qT")
            qkproj(xn0, qT0, [0, 1, 4])      # q pairs 0,1 + k pair 0
            vproj(xn0, vexts[0], [0, 1])
            pmem0 = [None, None]
            memsim(qT0, pmem0, 0)            # heads 0-3 (needs q0, q1)
            xn1, xn1_units = norm(1)         # stats now; xn tiles as fill

            attn0 = atp.tile([128, CT, N], BF16, tag="attn")
            attn1 = atp.tile([128, CT, N], BF16, tag="attn")
            pmem1 = [None, None]

            def place(fill, r0, units, per_round=1):
                r, i = r0, 0
                while i < len(units):
                    for _ in range(per_round):
                        if i < len(units):
                            fill[r].append(units[i])
                            i += 1
                    r += 1

            # batch-0 attention fill. kTp pair i is last read by batch-0 at
            # round 8*(2i+1)+7; vext[jc] is read by the av of round jc+2.
            f0 = [[] for _ in range(68)]
            place(f0, 0, [u for ic in range(2, 8) for u in vproj_units(xn0, vexts[0], ic)],
                  per_round=2)                             # v2-v7, rounds 0-5
            place(f0, 6, qkproj_units(xn0, qT0, 5))        # k1, read from rnd 16
            place(f0, 10, qkproj_units(xn0, qT0, 6))       # k2, read from rnd 32
            place(f0, 14, [xn1_units(t) for t in range(CT)])
            place(f0, 18, qkproj_units(xn0, qT0, 7))       # k3, read from rnd 48
            place(f0, 22, qkproj_units(xn0, qT0, 2))       # q pair 2
            place(f0, 26, qkproj_units(xn0, qT0, 3))       # q pair 3
            place(f0, 31, [memsim_unit(qT0, pmem0, 1)])    # heads 4-7, by rnd 41
            place(f0, 32, qkproj_units(xn1, qT1, 0))
            place(f0, 36, qkproj_units(xn1, qT1, 1))
            place(f0, 40, qkproj_units(xn1, qT1, 4))       # batch-1 k0 (safe: >15)
            place(f0, 44, qkproj_units(xn1, qT1, 2))
            place(f0, 48, qkproj_units(xn1, qT1, 3))
            place(f0, 52, [u for ic in range(8) for u in vproj_units(xn1, vexts[1], ic)],
                  per_round=2)                             # rounds 52-59
            place(f0, 60, [wo_cast_unit(t) for t in range(CT)])
            attention(qT0, vexts[0], attn0, pmem0, f0)

            # batch-1 attention fill: own k1-k3, then batch-0 out-proj.
            f1 = [[] for _ in range(68)]
            place(f1, 0, [memsim_unit(qT1, pmem1, 0), memsim_unit(qT1, pmem1, 1)])
            place(f1, 2, qkproj_units(xn1, qT1, 5))
            place(f1, 6, qkproj_units(xn1, qT1, 6))
            place(f1, 10, qkproj_units(xn1, qT1, 7))
            place(f1, 14, [u for mc in range(CT) for h2 in range(2)
                           for u in proj_units(attn0, 0, mc, h2)])
            attention(qT1, vexts[1], attn1, pmem1, f1)
            proj(attn1, 1)
    nc.compile()
    return nc


_NC_CACHE = []


def kernel(x, gamma, mem_kv, w_qkv, w_out, _trace=False):
    x = np.asarray(x, dtype=np.float32)
    gamma = np.asarray(gamma, dtype=np.float32)
    mem_kv = np.asarray(mem_kv, dtype=np.float32)
    w_qkv = np.asarray(w_qkv, dtype=np.float32)
    w_out = np.asarray(w_out, dtype=np.float32)

    b, c, hh, ww = x.shape
    n = hh * ww
    xs = x.reshape(b, c, n)

    wqkvt = np.ascontiguousarray(w_qkv.T)          # [c, 3c]
    wot = np.ascontiguousarray(w_out.T)            # [c, c]
    gammat = np.ascontiguousarray(gamma.reshape(CT, 128).T)  # [128, CT]

    memk = np.zeros((128, NPAIR, NMEM), np.float32)
    memv = np.zeros((128, 2, VW), np.float32)
    for h in range(HEADS):
        p, hh_ = h // 2, h % 2
        memk[64 * hh_:64 * hh_ + DH, p, 0:NMEM] = mem_kv[0, h].T  # [dh, nmem]
        g, r1, c0 = h // 4, 32 * (h % 4), (h % 4) * (DH + 1)
        memv[r1:r1 + NMEM, g, c0:c0 + DH] = mem_kv[1, h]
        memv[r1:r1 + NMEM, g, c0 + DH] = 1.0

    if not _NC_CACHE:
        _NC_CACHE.append(_build())
    nc = _NC_CACHE[0]

    in_maps = []
    for core in range(NCORES):
        in_maps.append({
            "x": np.ascontiguousarray(xs[core * PB:(core + 1) * PB]),
            "wqkvt": wqkvt,
            "wot": wot,
            "gammat": gammat,
            "memk": memk,
            "memv": memv,
        })
    res = run_bass_kernel_spmd(nc, in_maps, core_ids=list(range(NCORES)), trace=_trace)
    out = np.concatenate([res.results[core]["out"] for core in range(NCORES)], axis=0)
    kernel.last_result = res
    return out.reshape(b, c, hh, ww)
